# revision 2
# baseline (speedup 1.0000x reference)
"""Trainium2 Bass kernel for nn_GAT_87617332838818.

Mathematical collapse: the reference GAT's softmax weights sum to 1 within
each destination segment and the aggregated message ``hp[dst]`` is constant
within the segment, so message passing is the identity and the network is a
per-node 3-layer MLP:

    logits = W2r @ elu(W1r @ elu(W0r @ x^T))      (per node column)

with W0r = W0.reshape(96,128), W1r = W1.reshape(96,96), W2r = W2.reshape(40,96).

Device strategy (8 NeuronCores, node-sharded 6250 cols each), v2:
  - exact ELU with zero bias bookkeeping:  elu(p) = max(p,0) + (min(exp(p),1) - 1)
    r-pass:  DVE/ACT tensor_scalar max(p,0)        (PSUM read, 1x)
    e-pass:  ACT exp(p)                            (PSUM read, 1x)
    t'-pass: DVE (e min 1) add -1                  (SBUF fp16, 4x mode)
    both halves feed two accumulating matmuls (linearity), so no +1
    inflation ever exists and no per-layer bias corrections are needed.
  - supergroups of 1024 columns: ps0/ps1 PSUM tiles span 2 banks, so each
    drain pass covers 1024 cols in ONE instruction (halves instr count and
    the ~130ns/instr semaphore tax).  L2's [104,512] pair-packed output is
    written into ps1's bank 0 after the L1 drains release it, keeping the
    whole pipeline in exactly 8 PSUM banks (ps0 2x2 + ps1 2x2).
  - output drained as [104,512] (rows 0:40 = even 512-col group,
    64:104 = odd) then DMA'd as two clean [40,512] row-slices into a
    contiguous yT[40,6250] — no padded columns in the output DMA.
  - engine balance: exp is ACT-only; r/out drains are split ACT/DVE by
    static assignment sets tuned from traces.
  - warmup matmuls flip the PE p-state to 2.4 GHz during the DMA head.
"""

import os
import sys

import numpy as np

for _p in ("/root/.axon_site/_ro/trn_rl_repo", "/opt/trn_rl_repo"):
    if os.path.isdir(_p) and _p not in sys.path:
        sys.path.append(_p)

import concourse.bass as bass
import concourse.tile as tile
from concourse import bacc, mybir
from concourse.bass_utils import run_bass_kernel_spmd

N_CORES = 8
N_PER = 6250            # 50000 / 8
D_IN = 128
D_HID = 96
D_OUT = 40
BANK = 512              # matmul free-dim limit (1 PSUM bank of f32)
SGW = 1024              # supergroup width (2 PSUM banks)

F16 = mybir.dt.float16
F32 = mybir.dt.float32

Act = mybir.ActivationFunctionType
Alu = mybir.AluOpType

# supergroups: (start_col, width)
SGS = []
_c = 0
while _c < N_PER:
    SGS.append((_c, min(SGW, N_PER - _c)))
    _c += SGW
NSG = len(SGS)          # 7: six 1024-wide + one 106-wide tail

N_WARMUP_MM = 14        # dummy matmuls to flip the PE p-state
# which r-drains run on ACT instead of DVE, by (sg, layer)
R_ON_ACT = tuple((s, 1) for s in range(NSG) if s % 2 == 0)
# which out-drains run on ACT instead of DVE
OUT_ON_ACT = tuple(s for s in range(NSG) if s % 2 == 1)

# input DMA batches, by supergroup count
X_BATCHES = [1, 2, 2, 2]
_batch_of = {}
_b0 = 0
for _bi, _bn in enumerate(X_BATCHES):
    for _g in range(_b0, min(_b0 + _bn, NSG)):
        _batch_of[_g] = _bi
    _b0 += _bn
assert _b0 >= NSG


def _splits(w):
    out = []
    j = 0
    while j < w:
        out.append((j, min(j + BANK, w)))
        j += BANK
    return out


def _build_program() -> bass.Bass:
    nc = bacc.Bacc(None, target_bir_lowering=False, debug=False)

    # xw packs [w0t | xT]: cols 0..95 = W0^T fp16, cols 96.. = x^T shard
    xw = nc.declare_dram_parameter("xw", [D_IN, D_HID + N_PER], F16,
                                   isOutput=False)
    # wb packs [w1t | w2t] fp16
    wb = nc.declare_dram_parameter("wb", [D_HID, D_HID + D_OUT], F16,
                                   isOutput=False)
    yT = nc.declare_dram_parameter("yT", [D_OUT, N_PER], F16, isOutput=True)

    st = {}
    st_batch = {}
    shared = {}

    with tile.TileContext(nc) as tc:
        with (
            tc.tile_pool(name="consts", bufs=1) as consts,
            tc.tile_pool(name="x0", bufs=1) as x0pool,
            tc.tile_pool(name="xin", bufs=2) as xpool,
            tc.tile_pool(name="sb", bufs=2) as sb,
            tc.tile_pool(name="ps0", bufs=2, space="PSUM") as ps0,
            tc.tile_pool(name="ps1", bufs=2, space="PSUM") as ps1,
        ):
            # --- PE warm-up on garbage SBUF during the DMA-bound head.
            junk_w = consts.tile([D_IN, D_OUT], F16, tag="junkw")
            junk_x = consts.tile([D_IN, BANK], F16, tag="junkx")
            nc.gpsimd.memset(junk_w[:], 0.0)
            nc.gpsimd.memset(junk_x[:], 0.0)
            warm = ps1.tile([104, SGW], F32, tag="p1")
            for _ in range(N_WARMUP_MM):
                nc.tensor.matmul(warm[:D_OUT, :BANK], junk_w[:], junk_x[:],
                                 start=True, stop=True)

            wb_sb = consts.tile([D_HID, D_HID + D_OUT], F16, tag="wb")
            w1_sb = wb_sb[:, :D_HID]
            w2_sb = wb_sb[:, D_HID:D_HID + D_OUT]

            def rdrain(out_ap, psum_ap, on_act):
                """out = max(psum, 0), PSUM -> SBUF fp16."""
                if on_act:
                    nc.scalar.activation(out_ap, psum_ap, Act.Relu)
                else:
                    nc.vector.tensor_scalar_max(out_ap, psum_ap, 0.0)

            def stage_load(s):
                bi = _batch_of[s]
                if s > 0 and _batch_of[s - 1] == bi:
                    st[s] = st_batch[bi]
                    return
                s1 = s
                while s1 + 1 < NSG and _batch_of[s1 + 1] == bi:
                    s1 += 1
                lo = SGS[s][0] + (0 if bi else -D_HID)   # batch 0 incl. w0
                hi = SGS[s1][0] + SGS[s1][1]
                cols = hi - lo
                pool = x0pool if bi == 0 else xpool
                width = D_HID + SGW * X_BATCHES[0] if bi == 0 else SGW * 2
                xt = pool.tile([D_IN, width], F16,
                               tag=("xt0" if bi == 0 else "xt"))
                nc.sync.dma_start(xt[:, :cols], xw[:, D_HID + lo:D_HID + hi])
                st_batch[bi] = {"xt": xt, "base": lo}
                st[s] = st_batch[bi]

            def stage0(s):
                """L0 matmuls + L0 drains (exp0, r0, t0')."""
                c0, w = SGS[s]
                d = dict(st[s])
                st[s] = d
                xo = c0 - d["base"]
                w0_sb = shared["w0"]
                p0 = ps0.tile([D_HID, SGW], F32, tag="p0")
                for j0, j1 in _splits(w):
                    nc.tensor.matmul(p0[:, j0:j1], w0_sb,
                                     d["xt"][:, xo + j0:xo + j1],
                                     start=True, stop=True)
                e0 = sb.tile([D_HID, SGW], F16, tag="e0")
                r0 = sb.tile([D_HID, SGW], F16, tag="r0")
                t0 = sb.tile([D_HID, SGW], F16, tag="t0")
                nc.scalar.activation(e0[:, :w], p0[:, :w], Act.Exp)
                rdrain(r0[:, :w], p0[:, :w], (s, 0) in R_ON_ACT)
                nc.vector.tensor_scalar(t0[:, :w], e0[:, :w], 1.0, -1.0,
                                        Alu.min, Alu.add)
                d["r0"], d["t0"] = r0, t0

            def stage1(s):
                """L1 matmuls + L1 drains (exp1, r1, t1')."""
                w = SGS[s][1]
                d = st[s]
                p1 = ps1.tile([104, SGW], F32, tag="p1")
                for j0, j1 in _splits(w):
                    nc.tensor.matmul(p1[:D_HID, j0:j1], w1_sb,
                                     d["r0"][:, j0:j1], start=True, stop=False)
                    nc.tensor.matmul(p1[:D_HID, j0:j1], w1_sb,
                                     d["t0"][:, j0:j1], start=False, stop=True)
                e1 = sb.tile([D_HID, SGW], F16, tag="e1")
                r1 = sb.tile([D_HID, SGW], F16, tag="r1")
                t1 = sb.tile([D_HID, SGW], F16, tag="t1")
                nc.scalar.activation(e1[:, :w], p1[:D_HID, :w], Act.Exp)
                rdrain(r1[:, :w], p1[:D_HID, :w], (s, 1) in R_ON_ACT)
                nc.vector.tensor_scalar(t1[:, :w], e1[:, :w], 1.0, -1.0,
                                        Alu.min, Alu.add)
                d["r1"], d["t1"] = r1, t1
                d["p1"] = p1

            def stage2(s):
                """L2 matmuls into ps1 bank0 (pair-packed rows), drain, DMA."""
                c0, w = SGS[s]
                d = st.pop(s)
                p1 = d["p1"]
                w0_ = min(w, BANK)
                nc.tensor.matmul(p1[:D_OUT, :w0_], w2_sb, d["r1"][:, :w0_],
                                 start=True, stop=False)
                nc.tensor.matmul(p1[:D_OUT, :w0_], w2_sb, d["t1"][:, :w0_],
                                 start=False, stop=True)
                if w > BANK:
                    w1_ = w - BANK
                    nc.tensor.matmul(p1[64:64 + D_OUT, :w1_], w2_sb,
                                     d["r1"][:, BANK:w], start=True,
                                     stop=False)
                    nc.tensor.matmul(p1[64:64 + D_OUT, :w1_], w2_sb,
                                     d["t1"][:, BANK:w], start=False,
                                     stop=True)
                nrows = 104 if w > BANK else D_OUT
                ow = min(w, BANK)
                o = sb.tile([104, BANK], F16, tag="o")
                if s in OUT_ON_ACT:
                    nc.scalar.activation(o[:nrows, :ow], p1[:nrows, :ow],
                                         Act.Identity)
                else:
                    nc.vector.tensor_copy(o[:nrows, :ow], p1[:nrows, :ow])
                nc.sync.dma_start(yT[:, c0:c0 + w0_], o[:D_OUT, :w0_])
                if w > BANK:
                    nc.sync.dma_start(yT[:, c0 + BANK:c0 + w],
                                      o[64:64 + D_OUT, :w - BANK])

            for step in range(NSG + 2):
                if step < NSG:
                    stage_load(step)
                    if step == 0:
                        shared["w0"] = st[0]["xt"][:, 0:D_HID]
                        nc.sync.dma_start(wb_sb[:], wb[:])
                if 0 <= step - 0 < NSG and step < NSG:
                    stage0(step)
                if 0 <= step - 1 < NSG:
                    stage1(step - 1)
                if 0 <= step - 2 < NSG:
                    stage2(step - 2)

    nc.compile()
    return nc


_prog_cache = []
last_result = None


def kernel(**inputs) -> np.ndarray:
    global last_result
    x = np.asarray(inputs["x"], np.float32)           # [50000, 128]
    W0 = np.asarray(inputs["W0"], np.float32).reshape(D_HID, D_IN)
    W1 = np.asarray(inputs["W1"], np.float32).reshape(D_HID, D_HID)
    W2 = np.asarray(inputs["W2"], np.float32).reshape(D_OUT, D_HID)

    n = x.shape[0]
    assert n == N_CORES * N_PER, f"unexpected node count {n}"

    xT16 = x.T.astype(np.float16)                            # [128, 50000]
    w0t = W0.T.astype(np.float16)                            # [128, 96]
    wbm = np.ascontiguousarray(np.concatenate(
        [W1.T.astype(np.float16), W2.T.astype(np.float16)], axis=1))

    if not _prog_cache:
        _prog_cache.append(_build_program())
    nc = _prog_cache[0]

    in_maps = []
    for i in range(N_CORES):
        xwi = np.ascontiguousarray(
            np.concatenate([w0t, xT16[:, i * N_PER:(i + 1) * N_PER]], axis=1))
        in_maps.append(dict(xw=xwi, wb=wbm))
    res = run_bass_kernel_spmd(nc, in_maps, list(range(N_CORES)))
    last_result = res
    out = np.empty((n, D_OUT), np.float32)
    for i in range(N_CORES):
        yt = np.asarray(res.results[i]["yT"], np.float32)  # [40, 6250]
        out[i * N_PER:(i + 1) * N_PER] = yt.T
    return out


if __name__ == "__main__":
    data = np.load("/tmp/gat_inputs.npz")
    y = kernel(**{k: data[k] for k in data.files})
    print("out", y.shape, y.dtype, "absmax", np.abs(y).max())
